# revision 5
# baseline (speedup 1.0000x reference)
"""NT-Xent (contrastive) loss kernel for Trainium2, 8 NeuronCores.

Symmetric-blocked scheme: sim = (zn zn^T)/TEMP is symmetric, so only the
upper triangle of the 16x16 grid of 512x512 blocks is computed (136 blocks
over 8 cores = 17 each).  Each computed block (A, B) contributes
  - row-sums of exp(sim_block)  -> rows of strip A  (scalar-engine accum)
  - col-sums of exp(sim_block)  -> rows of strip B  (PE ones-matmul)
halving both the scalar-engine exp work and the PE matmul work vs the
row-sharded baseline.

All 8 cores run ONE program (SPMD); per-core work assignment comes from a
host-side permutation of the 512-row strips of z = concat(z_i, z_j):
the program always computes blocks (local strip 0 x locals 0..11) and
(local strip 12 x locals 11..15); a star decomposition of all 136
unordered strip pairs (8 stars of 12 edges + 8 stars of 5) defines the
per-core local->global strip map (duplicate strips allowed).

Device per core:
  - normalize all rows (scale 1/sqrt(TEMP*|z|^2) via DVE + ACT ln/exp)
  - transpose zn to feature-major znT via the DMA xbar (off the PE!)
  - 24 (chunk, m) iterations: 2-3 matmuls [128,512] -> psum, one big
    Exp with accum_out (row-sums), selector-matmul col-sums into one
    shared psum bank
  - positive pairs: row-dots of 512 (r, r+4096) pairs per core
Host: maps row/col partial sums back to global rows, S_r = sum, subtracts
the diagonal exp(2), lse = log(S), loss = (sum lse - sum pos)/2N.
"""

import sys

import numpy as np

if "/opt/trn_rl_repo" not in sys.path:
    sys.path.insert(0, "/opt/trn_rl_repo")

TWO_N = 8192
DIM = 128
N_CORES = 8
TEMP = 0.5
NSTRIP = 16
SW = 512  # strip width (rows)

# Block positions: C1 = (row local 0, col local j) j=0..11;
# C2 = (row local 12, col local j) j=11..15.
BLOCK_COL = list(range(12)) + list(range(11, 16))  # position -> col local
DIAG_POS = (0, 13)  # positions whose block is a diagonal block
CHUNKS = [[0, 1, 2], [3, 4, 5], [6, 7, 8], [9, 10, 11], [12, 13, 14], [15, 16]]
SELROW = {}
for _p in range(17):
    if _p not in DIAG_POS:
        SELROW[_p] = len(SELROW)  # 15 colsum rows


def _core_edges(c):
    ll = [(c + 1) % 8, (c + 2) % 8, (c + 3) % 8] + ([(c + 4) % 8] if c < 4 else [])
    cross_a = [8 + k for k in range(8) if not (c < 4 and k == c)]
    a_partners = ll + cross_a
    hh = [8 + (c + 1) % 8, 8 + (c + 2) % 8, 8 + (c + 3) % 8] + (
        [8 + (c - 4)] if c >= 4 else []
    )
    b_partners = hh + ([c] if c < 4 else [])
    return c, a_partners, 8 + c, b_partners


def _local_map(c):
    """16 local strip slots -> global strip ids (duplicates allowed)."""
    a, ap, b, bp = _core_edges(c)
    shared = 8 + (c + 1) % 8
    rest_a = [p for p in ap if p != shared]
    rest_b = [p for p in bp if p != shared]
    return [a] + rest_a + [shared] + [b] + rest_b


LOCAL_MAPS = [_local_map(c) for c in range(N_CORES)]


def _build():
    from contextlib import ExitStack

    import concourse.bass as bass
    import concourse.tile as tile
    from concourse import bacc, mybir

    f32 = mybir.dt.float32
    bf16 = mybir.dt.bfloat16
    AF = mybir.ActivationFunctionType

    nc = bacc.Bacc("TRN2", target_bir_lowering=False, debug=False)
    z_loc = nc.dram_tensor("z_loc", [TWO_N, DIM], f32, kind="ExternalInput").ap()
    z_pos = nc.dram_tensor("z_pos", [2 * SW, DIM], f32, kind="ExternalInput").ap()
    row_out = nc.dram_tensor("row_out", [128, 24], f32, kind="ExternalOutput").ap()
    col_out = nc.dram_tensor("col_out", [16, SW], f32, kind="ExternalOutput").ap()
    pos_out = nc.dram_tensor("pos_out", [128, 4], f32, kind="ExternalOutput").ap()

    n_colsum_mm = len(SELROW) * 4  # 60 accumulating colsum matmuls

    with tile.TileContext(nc) as tc, ExitStack() as ctx:
        const_pool = ctx.enter_context(tc.tile_pool(name="const", bufs=1))
        ld_pool = ctx.enter_context(tc.tile_pool(name="ld", bufs=3))
        zn_pool = ctx.enter_context(tc.tile_pool(name="zn", bufs=3))
        stat_pool = ctx.enter_context(tc.tile_pool(name="stat", bufs=3))
        sq_pool = ctx.enter_context(tc.tile_pool(name="sq", bufs=2))
        tpose_pool = ctx.enter_context(tc.tile_pool(name="tpose", bufs=1))
        es_pool = ctx.enter_context(tc.tile_pool(name="es", bufs=2))
        out_pool = ctx.enter_context(tc.tile_pool(name="outs", bufs=1))
        mm_psum = ctx.enter_context(tc.tile_pool(name="mmp", bufs=2, space="PSUM"))
        cs_psum = ctx.enter_context(tc.tile_pool(name="csp", bufs=1, space="PSUM"))
        acc_psum = ctx.enter_context(tc.tile_pool(name="accp", bufs=1, space="PSUM"))

        # Selector strip: zeros except column 15 = 1.0.  sel_j (ones in
        # column j of a [128,16] window) = sel[:, 15-j : 31-j].
        sel = const_pool.tile([128, 31], bf16, tag="sel")
        nc.vector.memset(sel[:], 0.0)
        nc.vector.memset(sel[:, 15:16], 1.0)

        # One transposed tile per 1024-row group: separate tiles so the
        # xbar-transpose writes don't serialize against main-loop reads.
        znT = [
            tpose_pool.tile([128, 1024], bf16, tag=f"znT{g}", name=f"znT{g}")
            for g in range(8)
        ]
        acc = acc_psum.tile([128, 24], f32, tag="acc")
        cs = cs_psum.tile([16, SW], f32, tag="cs")

        def normalize_group(z_src, dst2d, rows=1024):
            """Load `rows` rows of z_src, normalize by 1/sqrt(TEMP*|z|^2),
            write bf16 into dst2d [128, rows] (row r -> partition r%128,
            free col (r//128)*128 + d ... wait: layout [p, (a f)] with
            row = a*128 + p, feature f)."""
            na = rows // 128
            zt = ld_pool.tile([128, rows], f32, tag="ld")
            nc.sync.dma_start(
                zt[:].rearrange("p (a f) -> p a f", f=DIM),
                z_src.rearrange("(a p) f -> p a f", p=128),
            )
            sqw = sq_pool.tile([128, rows], bf16, tag="sq")
            nc.vector.tensor_mul(sqw[:], zt[:], zt[:])
            ssq = stat_pool.tile([128, na], f32, tag="ssq")
            nc.vector.reduce_sum(
                ssq[:],
                sqw[:].rearrange("p (a f) -> p a f", f=DIM),
                axis=mybir.AxisListType.X,
            )
            lnt = stat_pool.tile([128, na], f32, tag="lnt")
            nc.scalar.activation(lnt[:], ssq[:], AF.Ln, scale=float(TEMP))
            rn = stat_pool.tile([128, na], f32, tag="rn")
            nc.scalar.activation(rn[:], lnt[:], AF.Exp, scale=-0.5)
            for a in range(na):
                nc.vector.tensor_scalar_mul(
                    dst2d[:, a * DIM : (a + 1) * DIM],
                    zt[:, a * DIM : (a + 1) * DIM],
                    rn[:, a : a + 1],
                )

        # --- Startup: per 1024-row group: normalize + xbar-transpose ----
        for g in range(8):
            zng = zn_pool.tile([128, 1024], bf16, tag="zng")
            normalize_group(z_loc[g * 1024 : (g + 1) * 1024, :], zng)
            # znT[g][d, a*128 + p] = zng[p, a*128 + d]
            nc.sync.dma_start(
                znT[g][:].rearrange("d (a p) -> d a p", p=128),
                zng[:],
                transpose=True,
            )

        # --- Main loop ------------------------------------------------
        def znt_strip(s):
            """znT slice for local strip s (512 cols)."""
            return znT[s // 2][:, (s % 2) * SW : (s % 2 + 1) * SW]

        csmm = 0
        for ci, chunk in enumerate(CHUNKS):
            rstrip = 0 if ci < 4 else 12
            w = SW * len(chunk)
            for m in range(4):
                pt = mm_psum.tile([128, w], f32, tag="mm", padded_shape=[128, 1536])
                for k, p in enumerate(chunk):
                    nc.tensor.matmul(
                        pt[:, k * SW : (k + 1) * SW],
                        lhsT=znt_strip(rstrip)[:, m * 128 : (m + 1) * 128],
                        rhs=znt_strip(BLOCK_COL[p]),
                        start=True,
                        stop=True,
                    )
                es = es_pool.tile([128, w], bf16, tag="es", padded_shape=[128, 1536])
                col = ci * 4 + m
                nc.scalar.activation(
                    es[:], pt[:], AF.Exp, accum_out=acc[:, col : col + 1]
                )
                for k, p in enumerate(chunk):
                    if p in DIAG_POS:
                        continue
                    j = SELROW[p]
                    nc.tensor.matmul(
                        cs[:],
                        lhsT=sel[:, 15 - j : 31 - j],
                        rhs=es[:, k * SW : (k + 1) * SW],
                        start=(csmm == 0),
                        stop=(csmm == n_colsum_mm - 1),
                        skip_group_check=True,
                    )
                    csmm += 1

        # --- Positive pairs (scheduled into main-loop idle time) -------
        znp = zn_pool.tile([128, 1024], bf16, tag="znp")
        normalize_group(z_pos, znp)
        prod = sq_pool.tile([128, SW], bf16, tag="prod")
        nc.vector.tensor_mul(prod[:], znp[:, 0:SW], znp[:, SW : 2 * SW])
        posv = out_pool.tile([128, 4], f32, tag="posv")
        nc.vector.reduce_sum(
            posv[:],
            prod[:].rearrange("p (a f) -> p a f", f=DIM),
            axis=mybir.AxisListType.X,
        )
        nc.sync.dma_start(pos_out, posv[:])

        # --- Epilogue --------------------------------------------------
        row_sb = out_pool.tile([128, 24], f32, tag="row_sb")
        nc.vector.tensor_copy(row_sb[:], acc[:])
        nc.sync.dma_start(row_out, row_sb[:])
        col_sb = out_pool.tile([16, SW], f32, tag="col_sb")
        nc.vector.tensor_copy(col_sb[:], cs[:])
        nc.sync.dma_start(col_out, col_sb[:])

    # Force Ln and Exp onto the single shared ACT table set (see baseline).
    import concourse.bacc as bacc_mod
    from concourse.hw_specs import get_activation_tables as _real_gat

    AFt = mybir.ActivationFunctionType

    def _gat_ln_exp_shared(arch):
        tabs = _real_gat(arch)
        out = {}
        for name, fns in tabs.items():
            if name != "natural_log_exp_and_others":
                fns = fns - {AFt.Ln, AFt.Exp}
            out[name] = fns
        return out

    bacc_mod.get_activation_tables = _gat_ln_exp_shared
    try:
        nc.compile()
    finally:
        bacc_mod.get_activation_tables = _real_gat
    return nc


_NC_CACHE = None


def _get_nc():
    global _NC_CACHE
    if _NC_CACHE is None:
        _NC_CACHE = _build()
    return _NC_CACHE


def make_in_maps(z_i: np.ndarray, z_j: np.ndarray):
    z = np.concatenate([z_i, z_j], axis=0).astype(np.float32)
    in_maps = []
    for c in range(N_CORES):
        lm = LOCAL_MAPS[c]
        z_l = np.concatenate(
            [z[lm[s] * SW : (lm[s] + 1) * SW] for s in range(NSTRIP)], axis=0
        )
        zp = np.concatenate(
            [z[c * SW : (c + 1) * SW], z[TWO_N // 2 + c * SW : TWO_N // 2 + (c + 1) * SW]],
            axis=0,
        )
        in_maps.append(
            {
                "z_loc": np.ascontiguousarray(z_l),
                "z_pos": np.ascontiguousarray(zp),
            }
        )
    return in_maps


def combine(results):
    """Host-side: map per-core partial sums to global rows, finish loss."""
    S = np.zeros(TWO_N, dtype=np.float64)
    pos_total = 0.0
    for c, r in enumerate(results):
        lm = LOCAL_MAPS[c]
        rows = r["row_out"].astype(np.float64)  # [128, 24]
        cols = r["col_out"].astype(np.float64)  # [16, 512]
        for ci in range(6):
            g = lm[0 if ci < 4 else 12]
            for m in range(4):
                S[g * SW + m * 128 : g * SW + (m + 1) * 128] += rows[:, ci * 4 + m]
        for p, j in SELROW.items():
            g = lm[BLOCK_COL[p]]
            S[g * SW : (g + 1) * SW] += cols[j]
        pos_total += 2.0 * r["pos_out"].astype(np.float64).sum()
    lse = np.log(S - np.exp(2.0))
    return np.float32((lse.sum() - pos_total) / TWO_N)


def kernel(z_i: np.ndarray, z_j: np.ndarray) -> np.ndarray:
    from concourse.bass_utils import run_bass_kernel_spmd

    nc = _get_nc()
    in_maps = make_in_maps(np.asarray(z_i), np.asarray(z_j))
    res = run_bass_kernel_spmd(nc, in_maps, core_ids=list(range(N_CORES)))
    return combine(res.results)


# revision 7
# speedup vs baseline: 1.0689x; 1.0689x over previous
"""NT-Xent (contrastive) loss kernel for Trainium2, 8 NeuronCores.

Symmetric-blocked scheme: sim = (zn zn^T)/TEMP is symmetric, so only the
upper triangle of the 16x16 grid of 512x512 blocks is computed (136 blocks
over 8 cores = 17 each).  Each computed block (A, B) contributes
  - row-sums of exp(sim_block)  -> rows of strip A  (scalar-engine accum)
  - col-sums of exp(sim_block)  -> rows of strip B  (PE ones-matmul)
halving both the scalar-engine exp work and the PE matmul work vs the
row-sharded baseline.

All 8 cores run ONE program (SPMD); per-core work assignment comes from a
host-side permutation of the 512-row strips of z = concat(z_i, z_j):
the program always computes blocks (local strip 0 x locals 0..11) and
(local strip 12 x locals 11..15); a star decomposition of all 136
unordered strip pairs (8 stars of 12 edges + 8 stars of 5) defines the
per-core local->global strip map (duplicate strips allowed).

Device per core:
  - normalize all rows (scale 1/sqrt(TEMP*|z|^2) via DVE + ACT ln/exp)
  - transpose zn to feature-major znT via the DMA xbar (off the PE!)
  - 24 (chunk, m) iterations: 2-3 matmuls [128,512] -> psum, one big
    Exp with accum_out (row-sums), selector-matmul col-sums into one
    shared psum bank
  - positive pairs: row-dots of 512 (r, r+4096) pairs per core
Host: maps row/col partial sums back to global rows, S_r = sum, subtracts
the diagonal exp(2), lse = log(S), loss = (sum lse - sum pos)/2N.
"""

import sys

import numpy as np

if "/opt/trn_rl_repo" not in sys.path:
    sys.path.insert(0, "/opt/trn_rl_repo")

TWO_N = 8192
DIM = 128
N_CORES = 8
TEMP = 0.5
NSTRIP = 16
SW = 512  # strip width (rows)

# Block positions: C1 = (row local 0, col local j) j=0..11;
# C2 = (row local 12, col local j) j=11..15.
BLOCK_COL = list(range(12)) + list(range(11, 16))  # position -> col local
DIAG_POS = (0, 13)  # positions whose block is a diagonal block
CHUNKS = [[0, 1, 2], [3, 4, 5], [6, 7, 8], [9, 10, 11], [12, 13, 14], [15, 16]]
SELROW = {}
for _p in range(17):
    if _p not in DIAG_POS:
        SELROW[_p] = len(SELROW)  # 15 colsum rows


def _core_edges(c):
    ll = [(c + 1) % 8, (c + 2) % 8, (c + 3) % 8] + ([(c + 4) % 8] if c < 4 else [])
    cross_a = [8 + k for k in range(8) if not (c < 4 and k == c)]
    a_partners = ll + cross_a
    hh = [8 + (c + 1) % 8, 8 + (c + 2) % 8, 8 + (c + 3) % 8] + (
        [8 + (c - 4)] if c >= 4 else []
    )
    b_partners = hh + ([c] if c < 4 else [])
    return c, a_partners, 8 + c, b_partners


def _local_map(c):
    """16 local strip slots -> global strip ids (duplicates allowed)."""
    a, ap, b, bp = _core_edges(c)
    shared = 8 + (c + 1) % 8
    rest_a = [p for p in ap if p != shared]
    rest_b = [p for p in bp if p != shared]
    return [a] + rest_a + [shared] + [b] + rest_b


LOCAL_MAPS = [_local_map(c) for c in range(N_CORES)]


def _build():
    from contextlib import ExitStack

    import concourse.bass as bass
    import concourse.tile as tile
    from concourse import bacc, mybir

    f32 = mybir.dt.float32
    bf16 = mybir.dt.bfloat16
    AF = mybir.ActivationFunctionType

    nc = bacc.Bacc("TRN2", target_bir_lowering=False, debug=False)
    z_loc = nc.dram_tensor("z_loc", [TWO_N, DIM], f32, kind="ExternalInput").ap()
    z_pos = nc.dram_tensor("z_pos", [2 * SW, DIM], f32, kind="ExternalInput").ap()
    row_out = nc.dram_tensor("row_out", [128, 24], f32, kind="ExternalOutput").ap()
    col_out = nc.dram_tensor("col_out", [16, SW], f32, kind="ExternalOutput").ap()
    pos_out = nc.dram_tensor("pos_out", [128, 4], f32, kind="ExternalOutput").ap()

    n_colsum_mm = len(SELROW) * 4  # 60 accumulating colsum matmuls

    with tile.TileContext(nc) as tc, ExitStack() as ctx:
        const_pool = ctx.enter_context(tc.tile_pool(name="const", bufs=1))
        ld_pool = ctx.enter_context(tc.tile_pool(name="ld", bufs=4))
        zn_pool = ctx.enter_context(tc.tile_pool(name="zn", bufs=9))
        stat_pool = ctx.enter_context(tc.tile_pool(name="stat", bufs=4))
        sq_pool = ctx.enter_context(tc.tile_pool(name="sq", bufs=3))
        tpose_pool = ctx.enter_context(tc.tile_pool(name="tpose", bufs=1))
        es_pool = ctx.enter_context(tc.tile_pool(name="es", bufs=2))
        out_pool = ctx.enter_context(tc.tile_pool(name="outs", bufs=1))
        mm_psum = ctx.enter_context(tc.tile_pool(name="mmp", bufs=2, space="PSUM"))
        cs_psum = ctx.enter_context(tc.tile_pool(name="csp", bufs=1, space="PSUM"))
        acc_psum = ctx.enter_context(tc.tile_pool(name="accp", bufs=1, space="PSUM"))

        # Selector strip: zeros except column 15 = 1.0.  sel_j (ones in
        # column j of a [128,16] window) = sel[:, 15-j : 31-j].
        sel = const_pool.tile([128, 31], bf16, tag="sel")
        nc.vector.memset(sel[:], 0.0)
        nc.vector.memset(sel[:, 15:16], 1.0)

        # One transposed tile per 1024-row group: separate tiles so the
        # xbar-transpose writes don't serialize against main-loop reads.
        znT = [
            tpose_pool.tile([128, 1024], bf16, tag=f"znT{g}", name=f"znT{g}")
            for g in range(8)
        ]
        acc = acc_psum.tile([128, 24], f32, tag="acc")
        cs = cs_psum.tile([16, SW], f32, tag="cs")

        def normalize_group(z_src, dst2d, rows=1024, sq_gpsimd=True):
            """Load `rows` rows of z_src (layout [p, (a f)], row = a*128+p),
            normalize by 1/sqrt(TEMP*|z|^2), write bf16 into dst2d.

            Squares run on the otherwise-idle GPSIMD engine; the scale is a
            single DVE tensor_mul with rn broadcast along the feature dim."""
            na = rows // 128
            zt = ld_pool.tile([128, rows], f32, tag="ld")
            nc.sync.dma_start(
                zt[:].rearrange("p (a f) -> p a f", f=DIM),
                z_src.rearrange("(a p) f -> p a f", p=128),
            )
            sqw = sq_pool.tile([128, rows], bf16, tag="sq")
            sq_eng = nc.gpsimd if sq_gpsimd else nc.vector
            sq_eng.tensor_mul(sqw[:], zt[:], zt[:])
            ssq = stat_pool.tile([128, na], f32, tag="ssq")
            nc.vector.reduce_sum(
                ssq[:],
                sqw[:].rearrange("p (a f) -> p a f", f=DIM),
                axis=mybir.AxisListType.X,
            )
            lnt = stat_pool.tile([128, na], f32, tag="lnt")
            nc.scalar.activation(lnt[:], ssq[:], AF.Ln, scale=float(TEMP))
            rn = stat_pool.tile([128, na], f32, tag="rn")
            nc.scalar.activation(rn[:], lnt[:], AF.Exp, scale=-0.5)
            rn_b = (
                rn[:]
                .rearrange("p (a o) -> p a o", o=1)
                .broadcast_to([128, na, DIM])
            )
            nc.vector.tensor_mul(
                dst2d[:].rearrange("p (a f) -> p a f", f=DIM),
                zt[:].rearrange("p (a f) -> p a f", f=DIM),
                rn_b,
            )

        # --- Startup: per 1024-row group: normalize + xbar-transpose ----
        for g in range(8):
            zng = zn_pool.tile([128, 1024], bf16, tag="zng")
            normalize_group(z_loc[g * 1024 : (g + 1) * 1024, :], zng)
            # znT[g][d, a*128 + p] = zng[p, a*128 + d]
            nc.sync.dma_start(
                znT[g][:].rearrange("d (a p) -> d a p", p=128),
                zng[:],
                transpose=True,
            )

        # --- Main loop ------------------------------------------------
        def znt_strip(s):
            """znT slice for local strip s (512 cols)."""
            return znT[s // 2][:, (s % 2) * SW : (s % 2 + 1) * SW]

        csmm = 0
        for ci, chunk in enumerate(CHUNKS):
            rstrip = 0 if ci < 4 else 12
            w = SW * len(chunk)
            for m in range(4):
                pt = mm_psum.tile([128, w], f32, tag="mm", padded_shape=[128, 1536])
                for k, p in enumerate(chunk):
                    nc.tensor.matmul(
                        pt[:, k * SW : (k + 1) * SW],
                        lhsT=znt_strip(rstrip)[:, m * 128 : (m + 1) * 128],
                        rhs=znt_strip(BLOCK_COL[p]),
                        start=True,
                        stop=True,
                    )
                es = es_pool.tile([128, w], bf16, tag="es", padded_shape=[128, 1536])
                col = ci * 4 + m
                nc.scalar.activation(
                    es[:], pt[:], AF.Exp, accum_out=acc[:, col : col + 1]
                )
                for k, p in enumerate(chunk):
                    if p in DIAG_POS:
                        continue
                    j = SELROW[p]
                    nc.tensor.matmul(
                        cs[:],
                        lhsT=sel[:, 15 - j : 31 - j],
                        rhs=es[:, k * SW : (k + 1) * SW],
                        start=(csmm == 0),
                        stop=(csmm == n_colsum_mm - 1),
                        skip_group_check=True,
                    )
                    csmm += 1

        # --- Positive pairs (scheduled into main-loop idle time) -------
        znp = zn_pool.tile([128, 1024], bf16, tag="znp")
        normalize_group(z_pos, znp)
        prod = sq_pool.tile([128, SW], bf16, tag="prod")
        nc.vector.tensor_mul(prod[:], znp[:, 0:SW], znp[:, SW : 2 * SW])
        posv = out_pool.tile([128, 4], f32, tag="posv")
        nc.vector.reduce_sum(
            posv[:],
            prod[:].rearrange("p (a f) -> p a f", f=DIM),
            axis=mybir.AxisListType.X,
        )
        nc.sync.dma_start(pos_out, posv[:])

        # --- Epilogue --------------------------------------------------
        row_sb = out_pool.tile([128, 24], f32, tag="row_sb")
        nc.vector.tensor_copy(row_sb[:], acc[:])
        nc.sync.dma_start(row_out, row_sb[:])
        col_sb = out_pool.tile([16, SW], f32, tag="col_sb")
        nc.vector.tensor_copy(col_sb[:], cs[:])
        nc.sync.dma_start(col_out, col_sb[:])

    # Force Ln and Exp onto the single shared ACT table set (see baseline).
    import concourse.bacc as bacc_mod
    from concourse.hw_specs import get_activation_tables as _real_gat

    AFt = mybir.ActivationFunctionType

    def _gat_ln_exp_shared(arch):
        tabs = _real_gat(arch)
        out = {}
        for name, fns in tabs.items():
            if name != "natural_log_exp_and_others":
                fns = fns - {AFt.Ln, AFt.Exp}
            out[name] = fns
        return out

    bacc_mod.get_activation_tables = _gat_ln_exp_shared
    try:
        nc.compile()
    finally:
        bacc_mod.get_activation_tables = _real_gat
    return nc


_NC_CACHE = None


def _get_nc():
    global _NC_CACHE
    if _NC_CACHE is None:
        _NC_CACHE = _build()
    return _NC_CACHE


def make_in_maps(z_i: np.ndarray, z_j: np.ndarray):
    z = np.concatenate([z_i, z_j], axis=0).astype(np.float32)
    in_maps = []
    for c in range(N_CORES):
        lm = LOCAL_MAPS[c]
        z_l = np.concatenate(
            [z[lm[s] * SW : (lm[s] + 1) * SW] for s in range(NSTRIP)], axis=0
        )
        zp = np.concatenate(
            [z[c * SW : (c + 1) * SW], z[TWO_N // 2 + c * SW : TWO_N // 2 + (c + 1) * SW]],
            axis=0,
        )
        in_maps.append(
            {
                "z_loc": np.ascontiguousarray(z_l),
                "z_pos": np.ascontiguousarray(zp),
            }
        )
    return in_maps


def combine(results):
    """Host-side: map per-core partial sums to global rows, finish loss."""
    S = np.zeros(TWO_N, dtype=np.float64)
    pos_total = 0.0
    for c, r in enumerate(results):
        lm = LOCAL_MAPS[c]
        rows = r["row_out"].astype(np.float64)  # [128, 24]
        cols = r["col_out"].astype(np.float64)  # [16, 512]
        for ci in range(6):
            g = lm[0 if ci < 4 else 12]
            for m in range(4):
                S[g * SW + m * 128 : g * SW + (m + 1) * 128] += rows[:, ci * 4 + m]
        for p, j in SELROW.items():
            g = lm[BLOCK_COL[p]]
            S[g * SW : (g + 1) * SW] += cols[j]
        pos_total += 2.0 * r["pos_out"].astype(np.float64).sum()
    lse = np.log(S - np.exp(2.0))
    return np.float32((lse.sum() - pos_total) / TWO_N)


def kernel(z_i: np.ndarray, z_j: np.ndarray) -> np.ndarray:
    from concourse.bass_utils import run_bass_kernel_spmd

    nc = _get_nc()
    in_maps = make_in_maps(np.asarray(z_i), np.asarray(z_j))
    res = run_bass_kernel_spmd(nc, in_maps, core_ids=list(range(N_CORES)))
    return combine(res.results)


# revision 12
# speedup vs baseline: 1.2700x; 1.1882x over previous
"""NT-Xent (contrastive) loss kernel for Trainium2, 8 NeuronCores.

Symmetric-blocked scheme: sim = (zn zn^T)/TEMP is symmetric, so only the
upper triangle of the 16x16 grid of 512x512 blocks is computed (136 blocks
over 8 cores = 17 each).  Each computed block (A, B) contributes
  - row-sums of exp(sim_block)  -> rows of strip A  (scalar-engine accum)
  - col-sums of exp(sim_block)  -> rows of strip B  (PE ones-matmul)
halving both the scalar-engine exp work and the PE matmul work vs the
row-sharded baseline.

All 8 cores run ONE program (SPMD); per-core work assignment comes from a
host-side permutation of the 512-row strips of z = concat(z_i, z_j):
the program always computes blocks (local strip 0 x locals 0..11) and
(local strip 12 x locals 11..15); a star decomposition of all 136
unordered strip pairs (8 stars of 12 edges + 8 stars of 5) defines the
per-core local->global strip map (duplicate strips allowed).

Device per core:
  - normalize all rows (scale 1/sqrt(TEMP*|z|^2) via DVE + ACT ln/exp)
  - transpose zn to feature-major znT via the DMA xbar (off the PE!)
  - 24 (chunk, m) iterations: 2-3 matmuls [128,512] -> psum, one big
    Exp with accum_out (row-sums), selector-matmul col-sums into one
    shared psum bank
  - positive pairs: row-dots of 512 (r, r+4096) pairs per core
Host: maps row/col partial sums back to global rows, S_r = sum, subtracts
the diagonal exp(2), lse = log(S), loss = (sum lse - sum pos)/2N.
"""

import sys

import numpy as np

if "/opt/trn_rl_repo" not in sys.path:
    sys.path.insert(0, "/opt/trn_rl_repo")

TWO_N = 8192
DIM = 128
N_CORES = 8
TEMP = 0.5
NSTRIP = 16
SW = 512  # strip width (rows)

# Block positions: C1 = (row local 0, col local j) j=0..11;
# C2 = (row local 12, col local j) j=11..15.
BLOCK_COL = list(range(12)) + list(range(11, 16))  # position -> col local
DIAG_POS = (0, 13)  # positions whose block is a diagonal block
CHUNKS = [[0, 1, 2], [3, 4, 5], [6, 7, 8], [9, 10, 11], [12, 13, 14], [15, 16]]
SELROW = {}
for _p in range(17):
    if _p not in DIAG_POS:
        SELROW[_p] = len(SELROW)  # 15 colsum rows


def _core_edges(c):
    ll = [(c + 1) % 8, (c + 2) % 8, (c + 3) % 8] + ([(c + 4) % 8] if c < 4 else [])
    cross_a = [8 + k for k in range(8) if not (c < 4 and k == c)]
    a_partners = ll + cross_a
    hh = [8 + (c + 1) % 8, 8 + (c + 2) % 8, 8 + (c + 3) % 8] + (
        [8 + (c - 4)] if c >= 4 else []
    )
    b_partners = hh + ([c] if c < 4 else [])
    return c, a_partners, 8 + c, b_partners


def _local_map(c):
    """16 local strip slots -> global strip ids (duplicates allowed)."""
    a, ap, b, bp = _core_edges(c)
    shared = 8 + (c + 1) % 8
    rest_a = [p for p in ap if p != shared]
    rest_b = [p for p in bp if p != shared]
    return [a] + rest_a + [shared] + [b] + rest_b


LOCAL_MAPS = [_local_map(c) for c in range(N_CORES)]


def _build():
    from contextlib import ExitStack

    import concourse.bass as bass
    import concourse.tile as tile
    from concourse import bacc, mybir

    f32 = mybir.dt.float32
    bf16 = mybir.dt.bfloat16
    AF = mybir.ActivationFunctionType

    nc = bacc.Bacc("TRN2", target_bir_lowering=False, debug=False)
    z_loc = nc.dram_tensor("z_loc", [TWO_N, DIM], f32, kind="ExternalInput").ap()
    z_pos = nc.dram_tensor("z_pos", [2 * SW, DIM], f32, kind="ExternalInput").ap()
    row_out = nc.dram_tensor("row_out", [128, 24], f32, kind="ExternalOutput").ap()
    col_out = nc.dram_tensor("col_out", [16, SW], f32, kind="ExternalOutput").ap()
    pos_out = nc.dram_tensor("pos_out", [128, 4], f32, kind="ExternalOutput").ap()

    n_colsum_mm = len(SELROW) * 4  # 60 accumulating colsum matmuls

    with tile.TileContext(nc) as tc, ExitStack() as ctx:
        const_pool = ctx.enter_context(tc.tile_pool(name="const", bufs=1))
        ld_pool = ctx.enter_context(tc.tile_pool(name="ld", bufs=9))
        zn_pool = ctx.enter_context(tc.tile_pool(name="zn", bufs=1))
        stat_pool = ctx.enter_context(tc.tile_pool(name="stat", bufs=4))
        sq_pool = ctx.enter_context(tc.tile_pool(name="sq", bufs=3))
        tpose_pool = ctx.enter_context(tc.tile_pool(name="tpose", bufs=1))
        es_pool = ctx.enter_context(tc.tile_pool(name="es", bufs=2))
        out_pool = ctx.enter_context(tc.tile_pool(name="outs", bufs=1))
        mm_psum = ctx.enter_context(tc.tile_pool(name="mmp", bufs=2, space="PSUM"))
        cs_psum = ctx.enter_context(tc.tile_pool(name="csp", bufs=1, space="PSUM"))
        acc_psum = ctx.enter_context(tc.tile_pool(name="accp", bufs=1, space="PSUM"))

        # Selector strip: zeros except column 15 = 1.0.  sel_j (ones in
        # column j of a [128,16] window) = sel[:, 15-j : 31-j].
        sel = const_pool.tile([128, 31], bf16, tag="sel")
        nc.vector.memset(sel[:], 0.0)
        nc.vector.memset(sel[:, 15:16], 1.0)

        # One transposed tile per 1024-row group: separate tiles so the
        # xbar-transpose writes don't serialize against main-loop reads.
        znT = [
            tpose_pool.tile([128, 1024], bf16, tag=f"znT{g}", name=f"znT{g}")
            for g in range(8)
        ]
        acc = acc_psum.tile([128, 24], f32, tag="acc")
        cs = cs_psum.tile([16, SW], f32, tag="cs")

        def load_group(z_src, rows=1024, tag="ld"):
            zt = ld_pool.tile([128, rows], f32, tag=tag, name="zt")
            nc.sync.dma_start(
                zt[:].rearrange("p (a f) -> p a f", f=DIM),
                z_src.rearrange("(a p) f -> p a f", p=128),
            )
            return zt

        def normalize_group(zt, dst2d, rows=1024, sq_gpsimd=True):
            """Normalize pre-loaded rows (layout [p, (a f)], row = a*128+p)
            by 1/sqrt(TEMP*|z|^2), write bf16 into dst2d.

            Squares can run on the otherwise-idle GPSIMD engine; the scale is
            a single DVE tensor_mul with rn broadcast along the feature dim."""
            na = rows // 128
            sqw = sq_pool.tile([128, rows], bf16, tag="sq")
            sq_eng = nc.gpsimd if sq_gpsimd else nc.vector
            sq_eng.tensor_mul(sqw[:], zt[:], zt[:])
            ssq = stat_pool.tile([128, na], f32, tag="ssq")
            nc.vector.reduce_sum(
                ssq[:],
                sqw[:].rearrange("p (a f) -> p a f", f=DIM),
                axis=mybir.AxisListType.X,
            )
            lnt = stat_pool.tile([128, na], f32, tag="lnt")
            nc.scalar.activation(lnt[:], ssq[:], AF.Ln, scale=float(TEMP))
            rn = stat_pool.tile([128, na], f32, tag="rn")
            nc.scalar.activation(rn[:], lnt[:], AF.Exp, scale=-0.5)
            rn_b = (
                rn[:]
                .rearrange("p (a o) -> p a o", o=1)
                .broadcast_to([128, na, DIM])
            )
            nc.vector.tensor_mul(
                dst2d[:].rearrange("p (a f) -> p a f", f=DIM),
                zt[:].rearrange("p (a f) -> p a f", f=DIM),
                rn_b,
            )

        # --- Startup + main loop, interleaved --------------------------
        # Normalize groups in batches and emit main-loop chunks as soon as
        # their strips are covered: the scalar/sync engine queues are
        # in-order, so late groups' rsqrt/xbar must not sit ahead of the
        # main-loop exps/DMAs in the queues.
        zts = [load_group(z_loc[g * 1024 : (g + 1) * 1024, :]) for g in range(8)]

        def do_group(g):
            zng = zn_pool.tile([128, 1024], bf16, tag=f"zng{g}", name="zng")
            normalize_group(zts[g], zng, sq_gpsimd=(g >= 2))
            # znT[g][d, a*128 + p] = zng[p, a*128 + d]
            nc.sync.dma_start(
                znT[g][:].rearrange("d (a p) -> d a p", p=128),
                zng[:],
                transpose=True,
            )

        def znt_strip(s):
            """znT slice for local strip s (512 cols)."""
            return znT[s // 2][:, (s % 2) * SW : (s % 2 + 1) * SW]

        csmm = 0

        def do_chunk(ci):
            nonlocal csmm
            chunk = CHUNKS[ci]
            rstrip = 0 if ci < 4 else 12
            w = SW * len(chunk)
            for m in range(4):
                pt = mm_psum.tile([128, w], f32, tag="mm", padded_shape=[128, 1536])
                for k, p in enumerate(chunk):
                    nc.tensor.matmul(
                        pt[:, k * SW : (k + 1) * SW],
                        lhsT=znt_strip(rstrip)[:, m * 128 : (m + 1) * 128],
                        rhs=znt_strip(BLOCK_COL[p]),
                        start=True,
                        stop=True,
                    )
                es = es_pool.tile([128, w], bf16, tag="es", padded_shape=[128, 1536])
                col = ci * 4 + m
                nc.scalar.activation(
                    es[:], pt[:], AF.Exp, accum_out=acc[:, col : col + 1]
                )
                for k, p in enumerate(chunk):
                    if p in DIAG_POS:
                        continue
                    j = SELROW[p]
                    nc.tensor.matmul(
                        cs[:],
                        lhsT=sel[:, 15 - j : 31 - j],
                        rhs=es[:, k * SW : (k + 1) * SW],
                        start=(csmm == 0),
                        stop=(csmm == n_colsum_mm - 1),
                        skip_group_check=True,
                    )
                    csmm += 1

        for g in range(4):
            do_group(g)
        do_chunk(0)
        do_chunk(1)
        do_group(4)
        do_group(5)
        do_chunk(2)
        do_chunk(3)
        do_group(6)
        do_group(7)
        do_chunk(4)
        do_chunk(5)

        # --- Positive pairs (scheduled into main-loop idle time) -------
        ztp = load_group(z_pos, tag="ldp")
        znp = zn_pool.tile([128, 1024], bf16, tag="znp")
        normalize_group(ztp, znp)
        prod = sq_pool.tile([128, SW], bf16, tag="prod")
        nc.vector.tensor_mul(prod[:], znp[:, 0:SW], znp[:, SW : 2 * SW])
        posv = out_pool.tile([128, 4], f32, tag="posv")
        nc.vector.reduce_sum(
            posv[:],
            prod[:].rearrange("p (a f) -> p a f", f=DIM),
            axis=mybir.AxisListType.X,
        )
        nc.sync.dma_start(pos_out, posv[:])

        # --- Epilogue --------------------------------------------------
        row_sb = out_pool.tile([128, 24], f32, tag="row_sb")
        nc.vector.tensor_copy(row_sb[:], acc[:])
        nc.sync.dma_start(row_out, row_sb[:])
        col_sb = out_pool.tile([16, SW], f32, tag="col_sb")
        nc.vector.tensor_copy(col_sb[:], cs[:])
        nc.sync.dma_start(col_out, col_sb[:])

    # Force Ln and Exp onto the single shared ACT table set (see baseline).
    import concourse.bacc as bacc_mod
    from concourse.hw_specs import get_activation_tables as _real_gat

    AFt = mybir.ActivationFunctionType

    def _gat_ln_exp_shared(arch):
        tabs = _real_gat(arch)
        out = {}
        for name, fns in tabs.items():
            if name != "natural_log_exp_and_others":
                fns = fns - {AFt.Ln, AFt.Exp}
            out[name] = fns
        return out

    bacc_mod.get_activation_tables = _gat_ln_exp_shared
    try:
        nc.compile()
    finally:
        bacc_mod.get_activation_tables = _real_gat
    return nc


_NC_CACHE = None


def _get_nc():
    global _NC_CACHE
    if _NC_CACHE is None:
        _NC_CACHE = _build()
    return _NC_CACHE


def make_in_maps(z_i: np.ndarray, z_j: np.ndarray):
    z = np.concatenate([z_i, z_j], axis=0).astype(np.float32)
    in_maps = []
    for c in range(N_CORES):
        lm = LOCAL_MAPS[c]
        z_l = np.concatenate(
            [z[lm[s] * SW : (lm[s] + 1) * SW] for s in range(NSTRIP)], axis=0
        )
        zp = np.concatenate(
            [z[c * SW : (c + 1) * SW], z[TWO_N // 2 + c * SW : TWO_N // 2 + (c + 1) * SW]],
            axis=0,
        )
        in_maps.append(
            {
                "z_loc": np.ascontiguousarray(z_l),
                "z_pos": np.ascontiguousarray(zp),
            }
        )
    return in_maps


def combine(results):
    """Host-side: map per-core partial sums to global rows, finish loss."""
    S = np.zeros(TWO_N, dtype=np.float64)
    pos_total = 0.0
    for c, r in enumerate(results):
        lm = LOCAL_MAPS[c]
        rows = r["row_out"].astype(np.float64)  # [128, 24]
        cols = r["col_out"].astype(np.float64)  # [16, 512]
        for ci in range(6):
            g = lm[0 if ci < 4 else 12]
            for m in range(4):
                S[g * SW + m * 128 : g * SW + (m + 1) * 128] += rows[:, ci * 4 + m]
        for p, j in SELROW.items():
            g = lm[BLOCK_COL[p]]
            S[g * SW : (g + 1) * SW] += cols[j]
        pos_total += 2.0 * r["pos_out"].astype(np.float64).sum()
    lse = np.log(S - np.exp(2.0))
    return np.float32((lse.sum() - pos_total) / TWO_N)


def kernel(z_i: np.ndarray, z_j: np.ndarray) -> np.ndarray:
    from concourse.bass_utils import run_bass_kernel_spmd

    nc = _get_nc()
    in_maps = make_in_maps(np.asarray(z_i), np.asarray(z_j))
    res = run_bass_kernel_spmd(nc, in_maps, core_ids=list(range(N_CORES)))
    return combine(res.results)
